# revision 26
# baseline (speedup 1.0000x reference)
"""AttentionV1 Trainium2 Bass kernel (v2).

Data-parallel over batch: 8 images -> 8 NeuronCores. Per core:
  qkv = W_qkv @ x               (1x1 conv, PE, bf16, channel blocks
                                 reordered [q0|k0|v0|qk1-stacked|v1])
  qkv = dwconv3x3(qkv)          (9-tap stencil: PE diag-matmuls for most
                                 blocks, DVE tensor_scalar+STT for the rest)
  qf = q*f, kf = k*f            (DVE, fused from PSUM for PE-share blocks)
  row norms                     (ACT Square accum_out)
  qfT/kfT                       (batched DMA xbar transpose, 3D out)
  G  = qf @ kf^T                (PE, accumulated in PSUM over all chunks)
  attn = softmax(G*rq*rk^T*temp) (small-tensor phase)
  MT = blockdiag(attn)^T @ W_proj^T
  out = MT^T @ v                (PE, fused attn-apply + projection)
"""
import sys

for _p in ("/opt/trn_rl_repo",):
    if _p not in sys.path:
        sys.path.insert(0, _p)

import numpy as np

import concourse.bass as bass
import concourse.bacc as bacc
import concourse.mybir as mybir
from concourse.tile import TileContext
from concourse.bass_utils import run_bass_kernel_spmd

F32 = mybir.dt.float32
BF16 = mybir.dt.bfloat16
AL = mybir.AluOpType
AF = mybir.ActivationFunctionType

C = 192          # channels
O = 576          # 3*C
H = 128
W = 128
N = H * W        # 16384
HEADS = 8
CH = 24          # channels per head
TR = 8           # rows per spatial tile
NT = H // TR     # 16 tiles
S = TR * W       # 1024 spatial elems per tile
PW = W + 2       # padded width
PR = TR + 2      # padded rows

# stencil blocks in reordered channel space: name, psz, wq col start
# q0: q ch 0-127, k0: k ch 0-127, v0: v ch 0-127,
# qk1: q ch 128-191 (p0-63) + k ch 128-191 (p64-127), v1: v ch 128-191
BLOCKS = [("q0", 128, 0), ("k0", 128, 128), ("v0", 128, 256),
          ("qk1", 128, 384), ("v1", 64, 512)]
NB = len(BLOCKS)

# stencil engine assignment: block name -> "PE", "DVE", or "SPLIT"
# (SPLIT: chunk 0 on PE into PSUM, chunk 1 on DVE into bf16 acc)
import os
_ASSIGN = os.environ.get("ST_ASSIGN", "q0:SPLIT,k0:DVE,v0:PE,qk1:PE,v1:PE")
_ASSIGN_MAP = dict(kv.split(":") for kv in _ASSIGN.split(","))


def stencil_engine(bname, t):
    return _ASSIGN_MAP[bname]


def build_nc():
    nc = bacc.Bacc()
    x_d = nc.declare_dram_parameter("x", [C, H, W], F32, isOutput=False)
    f_d = nc.declare_dram_parameter("f", [C, H, W], F32, isOutput=False)
    wq_d = nc.declare_dram_parameter("wq", [C, O], BF16, isOutput=False)   # reordered W_qkv^T
    taps_d = nc.declare_dram_parameter("taps", [128, NB * 9], F32, isOutput=False)
    diag128_d = nc.declare_dram_parameter("diag128", [128, 4 * 9 * 128], BF16,
                                          isOutput=False)
    diag64_d = nc.declare_dram_parameter("diag64", [64, 9 * 64], BF16,
                                         isOutput=False)
    wp_d = nc.declare_dram_parameter("wp", [C, C], BF16, isOutput=False)   # W_proj^T
    tmpf_d = nc.declare_dram_parameter("tmpf", [1, C], F32, isOutput=False)
    out_d = nc.declare_dram_parameter("out", [C, H, W], F32, isOutput=True)
    out2 = out_d.rearrange("c h w -> c (h w)")

    with TileContext(nc) as tc:
        with (
            tc.tile_pool(name="const", bufs=1) as cpool,
            tc.tile_pool(name="xin", bufs=3) as xpool,
            tc.tile_pool(name="fin", bufs=3) as fpool,
            tc.tile_pool(name="acc", bufs=2) as apool,
            tc.tile_pool(name="qf", bufs=3) as qfpool,
            tc.tile_pool(name="scr", bufs=1) as scrpool,
            tc.tile_pool(name="tr", bufs=2) as trpool,
            tc.tile_pool(name="fin2", bufs=1) as finpool,
            tc.tile_pool(name="outsb", bufs=3) as outpool,
            tc.tile_pool(name="mm", bufs=3, space="PSUM") as mmpsum,
            tc.tile_pool(name="st", bufs=2, space="PSUM") as stpsum,
            tc.tile_pool(name="gram", bufs=1, space="PSUM") as gpsum,
            tc.tile_pool(name="fps", bufs=1, space="PSUM") as fpsum,
        ):
            # ---- constants ----
            wq_sb = [cpool.tile([128, O], BF16, tag="wq0", name="wq0"),
                     cpool.tile([64, O], BF16, tag="wq1", name="wq1")]
            nc.sync.dma_start(out=wq_sb[0][:], in_=wq_d[0:128, :])
            nc.sync.dma_start(out=wq_sb[1][:], in_=wq_d[128:192, :])
            taps_sb = cpool.tile([128, NB * 9], F32, tag="taps", name="taps")
            nc.sync.dma_start(out=taps_sb[:], in_=taps_d[:])
            diag_sb = cpool.tile([128, 4 * 9 * 128], BF16, tag="diag", name="diag")
            nc.sync.dma_start(out=diag_sb[:], in_=diag128_d[:])
            diag64_sb = cpool.tile([64, 9 * 64], BF16, tag="diag64", name="diag64")
            nc.sync.dma_start(out=diag64_sb[:], in_=diag64_d[:])
            wp_sb = [cpool.tile([96, C], BF16, tag="wp0", name="wp0"),
                     cpool.tile([96, C], BF16, tag="wp1", name="wp1")]
            nc.sync.dma_start(out=wp_sb[0][:], in_=wp_d[0:96, :])
            nc.sync.dma_start(out=wp_sb[1][:], in_=wp_d[96:192, :])
            tmpf_sb = cpool.tile([1, C], F32, tag="tmpf", name="tmpf")
            nc.sync.dma_start(out=tmpf_sb[:], in_=tmpf_d[:])

            # persistent v (bf16)
            v_sb = [cpool.tile([128, N], BF16, tag="v0sb", name="v0sb"),
                    cpool.tile([64, N], BF16, tag="v1sb", name="v1sb")]
            # per-tile square-sum slots: q0, k0, qk1-stacked
            sq_sb = {nm: cpool.tile([128, NT], F32, tag=f"sq_{nm}", name=f"sq_{nm}")
                     for nm in ("q0", "k0", "qk1")}

            # padded stencil buffers: 3 rotating slots per block
            pbuf = {}
            for bname, psz, _ in BLOCKS:
                pbuf[bname] = [
                    cpool.tile([psz, PR * PW], BF16, tag=f"pb_{bname}{s_}",
                               name=f"pb_{bname}{s_}")
                    for s_ in range(3)]
                for s_ in range(3):
                    b3 = pbuf[bname][s_].rearrange("p (r w) -> p r w", w=PW)
                    nc.vector.memset(b3[:, :, 0:1], 0.0)
                    nc.vector.memset(b3[:, :, PW - 1:PW], 0.0)

            # gram psum (accumulated across whole image)
            g_ps = [gpsum.tile([128, C], F32, tag="g0", name="g0"),
                    gpsum.tile([64, C], F32, tag="g1", name="g1")]

            def emit_loads(t):
                r0 = t * TR
                xt = [xpool.tile([128, S], BF16, tag="x0", name="x0"),
                      xpool.tile([64, S], BF16, tag="x1", name="x1")]
                nc.gpsimd.dma_start(out=xt[0][:], in_=x_d[0:128, r0:r0 + TR, :])
                nc.gpsimd.dma_start(out=xt[1][:], in_=x_d[128:192, r0:r0 + TR, :])
                f0 = fpool.tile([128, S], BF16, tag="f0", name="f0")
                f1 = fpool.tile([128, S], BF16, tag="f1", name="f1")
                nc.gpsimd.dma_start(out=f0[:], in_=f_d[0:128, r0:r0 + TR, :])
                nc.gpsimd.dma_start(out=f1[0:64, :], in_=f_d[128:192, r0:r0 + TR, :])
                nc.gpsimd.dma_start(out=f1[64:128, :], in_=f_d[128:192, r0:r0 + TR, :])
                return xt, (f0, f1)

            def emit_qkv(t, xt):
                """qkv matmuls + ACT copies into padded buffers + halo copies."""
                slot = t % 3
                if t == 0:
                    for bname, psz, _ in BLOCKS:
                        b3 = pbuf[bname][0].rearrange("p (r w) -> p r w", w=PW)
                        nc.vector.memset(b3[:, 0:1, 1:1 + W], 0.0)
                for bname, psz, ms in BLOCKS:
                    b3 = pbuf[bname][slot].rearrange("p (r w) -> p r w", w=PW)
                    for ci in range(2):
                        ps = mmpsum.tile([psz, 512], F32, tag="mmps", name="mmps")
                        nc.tensor.matmul(ps[:], wq_sb[0][:, ms:ms + psz],
                                         xt[0][:, ci * 512:(ci + 1) * 512],
                                         start=True, stop=False)
                        nc.tensor.matmul(ps[:], wq_sb[1][:, ms:ms + psz],
                                         xt[1][:, ci * 512:(ci + 1) * 512],
                                         start=False, stop=True)
                        # rows 1+4ci .. 5+4ci of padded buffer
                        nc.scalar.activation(
                            b3[:, 1 + 4 * ci:5 + 4 * ci, 1:1 + W],
                            ps[:].rearrange("p (r w) -> p r w", w=W), AF.Copy)
                        if ci == 0 and t > 0:
                            # halo: prev buffer row 9 <- this tile image row 0
                            pb_prev = pbuf[bname][(t - 1) % 3].rearrange(
                                "p (r w) -> p r w", w=PW)
                            nc.gpsimd.tensor_copy(
                                pb_prev[:, PR - 1:PR, 1:1 + W],
                                b3[:, 1:2, 1:1 + W])
                        if ci == 1 and t < NT - 1:
                            # halo: next buffer row 0 <- this tile image row 7
                            pb_next = pbuf[bname][(t + 1) % 3].rearrange(
                                "p (r w) -> p r w", w=PW)
                            nc.gpsimd.tensor_copy(
                                pb_next[:, 0:1, 1:1 + W],
                                b3[:, PR - 2:PR - 1, 1:1 + W])
                if t == NT - 1:
                    for bname, psz, _ in BLOCKS:
                        b3 = pbuf[bname][slot].rearrange("p (r w) -> p r w", w=PW)
                        nc.vector.memset(b3[:, PR - 1:PR, 1:1 + W], 0.0)

            OFFS = [(dy, dx) for dy in (-1, 0, 1) for dx in (-1, 0, 1)]

            def emit_stencil(t, ft):
                """stencil + f-mult + squares + transposes + gram for tile t."""
                slot = t % 3
                f0, f1 = ft
                qf = {}
                # --- per block stencil ---
                for bi, (bname, psz, _) in enumerate(BLOCKS):
                    b3 = pbuf[bname][slot].rearrange("p (r w) -> p r w", w=PW)
                    eng = stencil_engine(bname, t)
                    is_v = bname in ("v0", "v1")
                    fsrc = f1 if bname == "qk1" else f0
                    if not is_v:
                        q = qfpool.tile([psz, S], BF16, tag=f"qf_{bname}",
                                        name=f"qf_{bname}")
                        qf[bname] = q

                    def pe_chunk(ci):
                        sp = stpsum.tile([psz, 512], F32, tag="stps", name="stps")
                        for ti, (dy, dx) in enumerate(OFFS):
                            win = b3[:, 1 + 4 * ci + dy:5 + 4 * ci + dy,
                                     1 + dx:1 + dx + W]
                            if bname == "v1":
                                dg = diag64_sb[:, ti * 64:(ti + 1) * 64]
                            else:
                                dg = diag_sb[:, (bi * 9 + ti) * 128:
                                             (bi * 9 + ti) * 128 + psz]
                            nc.tensor.matmul(sp[:], dg, win,
                                             start=(ti == 0), stop=(ti == 8))
                        if is_v:
                            vi = 0 if bname == "v0" else 1
                            nc.scalar.activation(
                                v_sb[vi][:, t * S + ci * 512:
                                         t * S + (ci + 1) * 512],
                                sp[:], AF.Copy)
                        else:
                            nc.vector.tensor_mul(
                                qf[bname][:, ci * 512:(ci + 1) * 512], sp[:],
                                fsrc[:, ci * 512:(ci + 1) * 512])

                    def dve_chunks(cis):
                        nrow = 4 * len(cis)
                        r0_ = 4 * cis[0]
                        if is_v:
                            vi = 0 if bname == "v0" else 1
                            acc = v_sb[vi][:, t * S + r0_ * W:
                                           t * S + (r0_ + nrow) * W]
                        else:
                            acc = apool.tile([psz, 512 * len(cis)], BF16,
                                             tag=f"acc_{bname}{cis[0]}",
                                             name=f"acc_{bname}")
                        a3 = acc.rearrange("p (r w) -> p r w", w=W)
                        # 4x-mode tap products + 2x-mode add tree
                        prods = []
                        for ti, (dy, dx) in enumerate(OFFS):
                            win = b3[:, 1 + r0_ + dy:1 + r0_ + nrow + dy,
                                     1 + dx:1 + dx + W]
                            tap = taps_sb[0:psz, bi * 9 + ti:bi * 9 + ti + 1]
                            if ti == 0:
                                nc.vector.tensor_scalar_mul(a3[:], win, tap)
                            elif ti % 2 == 1:
                                pr = apool.tile([psz, 512 * len(cis)], BF16,
                                                tag=f"pr_{bname}{cis[0]}",
                                                name=f"pr_{bname}")
                                p3 = pr.rearrange("p (r w) -> p r w", w=W)
                                nc.vector.tensor_scalar_mul(p3[:], win, tap)
                                prods.append(p3)
                            else:
                                nc.vector.scalar_tensor_tensor(
                                    p3[:], win, tap, p3[:],
                                    op0=AL.mult, op1=AL.add)
                                nc.vector.tensor_add(a3[:], a3[:], p3[:])
                        # ti=7 leaves an un-accumulated product pair? no:
                        # pattern: ti 1(mul into p3) 2(stt+add) 3(mul) 4(stt+add)
                        # 5(mul) 6(stt+add) 7(mul) 8(stt+add) -> all folded
                        if not is_v:
                            nc.vector.tensor_mul(
                                qf[bname][:, r0_ * W:(r0_ + nrow) * W],
                                acc[:], fsrc[:, r0_ * W:(r0_ + nrow) * W])

                    if eng == "PE":
                        pe_chunk(0)
                        pe_chunk(1)
                    elif eng == "DVE":
                        dve_chunks((0, 1))
                    else:  # SPLIT
                        pe_chunk(0)
                        dve_chunks((1,))

                # --- squares (ACT) ---
                for nm in ("q0", "k0", "qk1"):
                    scr = scrpool.tile([128, S], BF16, tag=f"scr_{nm}",
                                       name=f"scr_{nm}")
                    nc.scalar.activation(scr[:], qf[nm][:], AF.Square,
                                         accum_out=sq_sb[nm][:, t:t + 1])

                # --- transposes (batched xbar DMA) ---
                qT = {}
                for nm in ("q0", "k0", "qk1"):
                    tt = trpool.tile([128, TR * 128], BF16, tag=f"T_{nm}",
                                     name=f"T_{nm}")
                    t3 = tt.rearrange("p (j c) -> p j c", c=128)
                    nc.sync.dma_start_transpose(t3[:], qf[nm][:])
                    qT[nm] = t3

                # --- gram accumulation ---
                for j in range(TR):
                    st = (t == 0 and j == 0)
                    sp = (t == NT - 1 and j == TR - 1)
                    q0c = qT["q0"][:, j, :]
                    q1c = qT["qk1"][:, j, 0:64]
                    k0c = qT["k0"][:, j, :]
                    k1c = qT["qk1"][:, j, 64:128]
                    nc.tensor.matmul(g_ps[0][:, 0:128], q0c, k0c,
                                     start=st, stop=sp)
                    nc.tensor.matmul(g_ps[0][:, 128:192], q0c, k1c,
                                     start=st, stop=sp)
                    nc.tensor.matmul(g_ps[1][:, 0:128], q1c, k0c,
                                     start=st, stop=sp)
                    nc.tensor.matmul(g_ps[1][:, 128:192], q1c, k1c,
                                     start=st, stop=sp)

            # ================= main pipelined loop =================
            prev = None
            for t in range(NT):
                xt, ft = emit_loads(t)
                emit_qkv(t, xt)
                if prev is not None:
                    emit_stencil(prev[0], prev[1])
                prev = (t, ft)
            emit_stencil(prev[0], prev[1])

            # ================= small-tensor phase =================
            r_ = {}
            for nm in ("q0", "k0", "qk1"):
                s1 = finpool.tile([128, 1], F32, tag=f"s1_{nm}", name=f"s1_{nm}")
                nc.vector.tensor_reduce(s1[:], sq_sb[nm][:],
                                        axis=mybir.AxisListType.X, op=AL.add)
                nc.vector.tensor_scalar_max(s1[:], s1[:], 1e-24)
                sq1 = finpool.tile([128, 1], F32, tag=f"sr_{nm}", name=f"sr_{nm}")
                nc.scalar.activation(sq1[:], s1[:], AF.Sqrt)
                rr = finpool.tile([128, 1], F32, tag=f"r_{nm}", name=f"r_{nm}")
                nc.vector.reciprocal(rr[:], sq1[:])
                r_[nm] = rr

            G_sb = [finpool.tile([128, C], F32, tag="G0", name="G0"),
                    finpool.tile([64, C], F32, tag="G1", name="G1")]
            nc.vector.tensor_scalar_mul(G_sb[0][:], g_ps[0][:], r_["q0"][:])
            nc.vector.tensor_scalar_mul(G_sb[1][:], g_ps[1][:], r_["qk1"][0:64, :])

            rkf = finpool.tile([1, C], F32, tag="rkf", name="rkf")
            nc.gpsimd.dma_start(out=rkf[0:1, 0:128], in_=r_["k0"][:, 0:1])
            nc.gpsimd.dma_start(out=rkf[0:1, 128:192], in_=r_["qk1"][64:128, 0:1])
            nc.vector.tensor_mul(rkf[:], rkf[:], tmpf_sb[:])
            rkb = finpool.tile([128, C], F32, tag="rkb", name="rkb")
            nc.gpsimd.partition_broadcast(rkb[:], rkf[:])
            nc.vector.tensor_mul(G_sb[0][:], G_sb[0][:], rkb[0:128, :])
            nc.vector.tensor_mul(G_sb[1][:], G_sb[1][:], rkb[0:64, :])

            # extract per-head 24x24 blocks: at[c, h*24+d] = Gs[24h+c, 24h+d]
            at = finpool.tile([CH, HEADS * CH], F32, tag="at", name="at")
            for h in range(HEADS):
                a0 = h * CH
                col = slice(a0, a0 + CH)
                dst = at[:, a0:a0 + CH]
                if a0 + CH <= 128:
                    nc.sync.dma_start(out=dst, in_=G_sb[0][a0:a0 + CH, col])
                elif a0 >= 128:
                    nc.sync.dma_start(out=dst,
                                      in_=G_sb[1][a0 - 128:a0 - 128 + CH, col])
                else:
                    m = 128 - a0
                    nc.sync.dma_start(out=at[0:m, a0:a0 + CH],
                                      in_=G_sb[0][a0:128, col])
                    nc.sync.dma_start(out=at[m:CH, a0:a0 + CH],
                                      in_=G_sb[1][0:a0 + CH - 128, col])

            e_sb = finpool.tile([CH, HEADS * CH], F32, tag="e", name="e")
            nc.scalar.activation(e_sb[:], at[:], AF.Exp)
            e3 = e_sb.rearrange("p (h d) -> p h d", d=CH)
            sums = finpool.tile([CH, HEADS], F32, tag="sums", name="sums")
            nc.vector.tensor_reduce(sums[:], e3[:], axis=mybir.AxisListType.X,
                                    op=AL.add)
            rs = finpool.tile([CH, HEADS], F32, tag="rs", name="rs")
            nc.vector.reciprocal(rs[:], sums[:])
            attn = finpool.tile([CH, HEADS * CH], BF16, tag="attn", name="attn")
            for h in range(HEADS):
                nc.vector.tensor_scalar_mul(
                    attn[:, h * CH:(h + 1) * CH],
                    e_sb[:, h * CH:(h + 1) * CH], rs[:, h:h + 1])

            # blockdiag(attn) as two 96-row contraction blocks
            bd = [finpool.tile([96, C], BF16, tag="bd0", name="bd0"),
                  finpool.tile([96, C], BF16, tag="bd1", name="bd1")]
            nc.vector.memset(bd[0][:], 0.0)
            nc.vector.memset(bd[1][:], 0.0)
            for h in range(HEADS):
                nc.sync.dma_start(
                    out=bd[h // 4][(h % 4) * CH:(h % 4) * CH + CH,
                                   h * CH:(h + 1) * CH],
                    in_=attn[:, h * CH:(h + 1) * CH])
            mtps_all = fpsum.tile([128, 512], F32, tag="mtps", name="mtps")
            mt_ps = [mtps_all[:, 0:C], mtps_all[0:64, C:2 * C]]
            for mi, msl in enumerate((slice(0, 128), slice(128, 192))):
                for k in range(2):
                    nc.tensor.matmul(mt_ps[mi][:], bd[k][:, msl], wp_sb[k][:],
                                     start=(k == 0), stop=(k == 1))
            mt_sb = [finpool.tile([128, C], BF16, tag="mts0", name="mts0"),
                     finpool.tile([64, C], BF16, tag="mts1", name="mts1")]
            nc.vector.tensor_copy(mt_sb[0][:], mt_ps[0][:])
            nc.vector.tensor_copy(mt_sb[1][:], mt_ps[1][:])

            # ---- output: out = MT^T @ v ----
            for jj in range(N // 1024):
                osb = [outpool.tile([128, 1024], BF16, tag="osb0", name="osb0"),
                       outpool.tile([64, 1024], BF16, tag="osb1", name="osb1")]
                for half in range(2):
                    col = slice(jj * 1024 + half * 512, jj * 1024 + (half + 1) * 512)
                    ocol = slice(half * 512, (half + 1) * 512)
                    for mi, (msz, msl) in enumerate(((128, slice(0, 128)),
                                                     (64, slice(128, 192)))):
                        ps = mmpsum.tile([msz, 512], F32, tag="mmps", name="mmps")
                        nc.tensor.matmul(ps[:], mt_sb[0][:, msl],
                                         v_sb[0][:, col], start=True, stop=False)
                        nc.tensor.matmul(ps[:], mt_sb[1][:, msl],
                                         v_sb[1][:, col], start=False, stop=True)
                        if (jj + half + mi) % 2 == 0:
                            nc.scalar.activation(osb[mi][:, ocol], ps[:], AF.Copy)
                        else:
                            nc.vector.tensor_copy(osb[mi][:, ocol], ps[:])
                nc.gpsimd.dma_start(out=out2[0:128, jj * 1024:(jj + 1) * 1024],
                                    in_=osb[0][:])
                nc.gpsimd.dma_start(out=out2[128:192, jj * 1024:(jj + 1) * 1024],
                                    in_=osb[1][:])
    nc.finalize()
    return nc


_NC_CACHE = {}

# reordered qkv output channel index lists (in original 0..575 space)
_ORDER = (list(range(0, 128))          # q0
          + list(range(192, 320))      # k0
          + list(range(384, 512))      # v0
          + list(range(128, 192)) + list(range(320, 384))   # qk1 stacked
          + list(range(512, 576)))     # v1


def _prep_consts(W_qkv, W_dw, W_proj, temperature):
    import ml_dtypes
    order = np.array(_ORDER)
    wq = np.asarray(W_qkv, np.float32)[order, :]          # [576, 192] reordered
    wqT = np.ascontiguousarray(wq.T).astype(ml_dtypes.bfloat16)
    w9 = np.asarray(W_dw, np.float32).reshape(O, 9)[order, :]   # [576, 9]
    # taps param [128, 45]: col b*9+t = tap t of block b (on that block's partitions)
    taps = np.zeros((128, NB * 9), np.float32)
    starts = [0, 128, 256, 384, 512]
    for b in range(NB):
        psz = 64 if b == 4 else 128
        taps[0:psz, b * 9:(b + 1) * 9] = w9[starts[b]:starts[b] + psz, :]
    # diag matrices
    d128 = np.zeros((128, 4 * 9 * 128), np.float32)
    for b in range(4):
        for t in range(9):
            d128[:, (b * 9 + t) * 128:(b * 9 + t + 1) * 128] = np.diag(
                w9[starts[b]:starts[b] + 128, t])
    d64 = np.zeros((64, 9 * 64), np.float32)
    for t in range(9):
        d64[:, t * 64:(t + 1) * 64] = np.diag(w9[512:576, t])
    wp = np.ascontiguousarray(np.asarray(W_proj, np.float32).T).astype(
        ml_dtypes.bfloat16)
    tmpf = np.repeat(np.asarray(temperature, np.float32).reshape(HEADS), CH)
    return {
        "wq": wqT,
        "taps": np.ascontiguousarray(taps),
        "diag128": np.ascontiguousarray(d128.astype(ml_dtypes.bfloat16)),
        "diag64": np.ascontiguousarray(d64.astype(ml_dtypes.bfloat16)),
        "wp": wp,
        "tmpf": np.ascontiguousarray(tmpf.reshape(1, C)),
    }


def kernel(x, feature, W_qkv, W_dw, W_proj, temperature):
    b = x.shape[0]
    consts = _prep_consts(W_qkv, W_dw, W_proj, temperature)

    if "nc" not in _NC_CACHE:
        _NC_CACHE["nc"] = build_nc()
    nc = _NC_CACHE["nc"]

    in_maps = []
    for i in range(b):
        m = {"x": np.ascontiguousarray(np.asarray(x[i], np.float32)),
             "f": np.ascontiguousarray(np.asarray(feature[i], np.float32))}
        m.update(consts)
        in_maps.append(m)
    res = run_bass_kernel_spmd(nc, in_maps, list(range(b)))
    _NC_CACHE["last_result"] = res
    outs = [np.asarray(r["out"], np.float32).reshape(C, H, W)
            for r in res.results]
    return np.stack(outs, axis=0)


# revision 29
# speedup vs baseline: 1.0781x; 1.0781x over previous
"""AttentionV1 Trainium2 Bass kernel (v2).

Data-parallel over batch: 8 images -> 8 NeuronCores. Per core:
  qkv = W_qkv @ x               (1x1 conv, PE, bf16, channel blocks
                                 reordered [q0|k0|v0|qk1-stacked|v1])
  qkv = dwconv3x3(qkv)          (9-tap stencil: PE diag-matmuls for most
                                 blocks, DVE tensor_scalar+STT for the rest)
  qf = q*f, kf = k*f            (DVE, fused from PSUM for PE-share blocks)
  row norms                     (ACT Square accum_out)
  qfT/kfT                       (batched DMA xbar transpose, 3D out)
  G  = qf @ kf^T                (PE, accumulated in PSUM over all chunks)
  attn = softmax(G*rq*rk^T*temp) (small-tensor phase)
  MT = blockdiag(attn)^T @ W_proj^T
  out = MT^T @ v                (PE, fused attn-apply + projection)
"""
import sys

for _p in ("/opt/trn_rl_repo",):
    if _p not in sys.path:
        sys.path.insert(0, _p)

import numpy as np

import concourse.bass as bass
import concourse.bacc as bacc
import concourse.mybir as mybir
from concourse.tile import TileContext
from concourse.bass_utils import run_bass_kernel_spmd

F32 = mybir.dt.float32
BF16 = mybir.dt.bfloat16
AL = mybir.AluOpType
AF = mybir.ActivationFunctionType

C = 192          # channels
O = 576          # 3*C
H = 128
W = 128
N = H * W        # 16384
HEADS = 8
CH = 24          # channels per head
TR = 8           # rows per spatial tile
NT = H // TR     # 16 tiles
S = TR * W       # 1024 spatial elems per tile
PW = W + 2       # padded width
PR = TR + 2      # padded rows

# stencil blocks in reordered channel space: name, psz, wq col start
# q0: q ch 0-127, k0: k ch 0-127, v0: v ch 0-127,
# qk1: q ch 128-191 (p0-63) + k ch 128-191 (p64-127), v1: v ch 128-191
BLOCKS = [("q0", 128, 0), ("k0", 128, 128), ("v0", 128, 256),
          ("qk1", 128, 384), ("v1", 64, 512)]
NB = len(BLOCKS)

# stencil engine assignment: block name -> "PE", "DVE", or "SPLIT"
# (SPLIT: chunk 0 on PE into PSUM, chunk 1 on DVE into bf16 acc)
import os
_ASSIGN = os.environ.get("ST_ASSIGN", "q0:SPLIT,k0:DVE,v0:PE,qk1:PE,v1:PE")
_ASSIGN_MAP = dict(kv.split(":") for kv in _ASSIGN.split(","))


def stencil_engine(bname, t):
    return _ASSIGN_MAP[bname]


def build_nc():
    nc = bacc.Bacc()
    x_d = nc.declare_dram_parameter("x", [C, H, W], F32, isOutput=False)
    f_d = nc.declare_dram_parameter("f", [C, H, W], F32, isOutput=False)
    wq_d = nc.declare_dram_parameter("wq", [C, O], BF16, isOutput=False)   # reordered W_qkv^T
    taps_d = nc.declare_dram_parameter("taps", [128, NB * 9], F32, isOutput=False)
    diag128_d = nc.declare_dram_parameter("diag128", [128, 4 * 9 * 128], BF16,
                                          isOutput=False)
    diag64_d = nc.declare_dram_parameter("diag64", [64, 9 * 64], BF16,
                                         isOutput=False)
    wp_d = nc.declare_dram_parameter("wp", [C, C], BF16, isOutput=False)   # W_proj^T
    tmpf_d = nc.declare_dram_parameter("tmpf", [1, C], F32, isOutput=False)
    out_d = nc.declare_dram_parameter("out", [C, H, W], F32, isOutput=True)
    out2 = out_d.rearrange("c h w -> c (h w)")

    with TileContext(nc) as tc:
        with (
            tc.tile_pool(name="const", bufs=1) as cpool,
            tc.tile_pool(name="xin", bufs=3) as xpool,
            tc.tile_pool(name="fin", bufs=3) as fpool,
            tc.tile_pool(name="acc", bufs=2) as apool,
            tc.tile_pool(name="qf", bufs=3) as qfpool,
            tc.tile_pool(name="scr", bufs=1) as scrpool,
            tc.tile_pool(name="tr", bufs=2) as trpool,
            tc.tile_pool(name="fin2", bufs=1) as finpool,
            tc.tile_pool(name="outsb", bufs=2) as outpool,
            tc.tile_pool(name="mm", bufs=3, space="PSUM") as mmpsum,
            tc.tile_pool(name="st", bufs=2, space="PSUM") as stpsum,
            tc.tile_pool(name="gram", bufs=1, space="PSUM") as gpsum,
            tc.tile_pool(name="fps", bufs=1, space="PSUM") as fpsum,
        ):
            # ---- constants ----
            wq_sb = [cpool.tile([128, O], BF16, tag="wq0", name="wq0"),
                     cpool.tile([64, O], BF16, tag="wq1", name="wq1")]
            nc.sync.dma_start(out=wq_sb[0][:], in_=wq_d[0:128, :])
            nc.sync.dma_start(out=wq_sb[1][:], in_=wq_d[128:192, :])
            taps_sb = cpool.tile([128, NB * 9], F32, tag="taps", name="taps")
            nc.sync.dma_start(out=taps_sb[:], in_=taps_d[:])
            diag_sb = cpool.tile([128, 4 * 9 * 128], BF16, tag="diag", name="diag")
            nc.sync.dma_start(out=diag_sb[:], in_=diag128_d[:])
            diag64_sb = cpool.tile([64, 9 * 64], BF16, tag="diag64", name="diag64")
            nc.sync.dma_start(out=diag64_sb[:], in_=diag64_d[:])
            wp_sb = [cpool.tile([96, C], BF16, tag="wp0", name="wp0"),
                     cpool.tile([96, C], BF16, tag="wp1", name="wp1")]
            nc.sync.dma_start(out=wp_sb[0][:], in_=wp_d[0:96, :])
            nc.sync.dma_start(out=wp_sb[1][:], in_=wp_d[96:192, :])
            tmpf_sb = cpool.tile([1, C], F32, tag="tmpf", name="tmpf")
            nc.sync.dma_start(out=tmpf_sb[:], in_=tmpf_d[:])

            # prewarm ACT tables used in the final phase
            warm = cpool.tile([1, 2], F32, tag="warm", name="warm")
            nc.vector.memset(warm[:], 1.0)
            nc.scalar.activation(warm[:, 0:1], warm[:, 1:2], AF.Sqrt)
            nc.scalar.activation(warm[:, 0:1], warm[:, 1:2], AF.Exp)

            # persistent v (bf16)
            v_sb = [cpool.tile([128, N], BF16, tag="v0sb", name="v0sb"),
                    cpool.tile([64, N], BF16, tag="v1sb", name="v1sb")]
            # per-tile square-sum slots: q0, k0, qk1-stacked
            sq_sb = {nm: cpool.tile([128, NT], F32, tag=f"sq_{nm}", name=f"sq_{nm}")
                     for nm in ("q0", "k0", "qk1")}

            # padded stencil buffers: 3 rotating slots per block
            pbuf = {}
            for bname, psz, _ in BLOCKS:
                pbuf[bname] = [
                    cpool.tile([psz, PR * PW], BF16, tag=f"pb_{bname}{s_}",
                               name=f"pb_{bname}{s_}")
                    for s_ in range(3)]
                for s_ in range(3):
                    b3 = pbuf[bname][s_].rearrange("p (r w) -> p r w", w=PW)
                    nc.vector.memset(b3[:, :, 0:1], 0.0)
                    nc.vector.memset(b3[:, :, PW - 1:PW], 0.0)

            # gram psum (accumulated across whole image)
            g_ps = [gpsum.tile([128, C], F32, tag="g0", name="g0"),
                    gpsum.tile([64, C], F32, tag="g1", name="g1")]

            def emit_loads(t):
                r0 = t * TR
                xt = [xpool.tile([128, S], BF16, tag="x0", name="x0"),
                      xpool.tile([64, S], BF16, tag="x1", name="x1")]
                nc.gpsimd.dma_start(out=xt[0][:], in_=x_d[0:128, r0:r0 + TR, :])
                nc.gpsimd.dma_start(out=xt[1][:], in_=x_d[128:192, r0:r0 + TR, :])
                f0 = fpool.tile([128, S], BF16, tag="f0", name="f0")
                f1 = fpool.tile([128, S], BF16, tag="f1", name="f1")
                nc.gpsimd.dma_start(out=f0[:], in_=f_d[0:128, r0:r0 + TR, :])
                nc.gpsimd.dma_start(out=f1[0:64, :], in_=f_d[128:192, r0:r0 + TR, :])
                nc.gpsimd.dma_start(out=f1[64:128, :], in_=f_d[128:192, r0:r0 + TR, :])
                return xt, (f0, f1)

            def emit_qkv(t, xt):
                """qkv matmuls + ACT copies into padded buffers + halo copies."""
                slot = t % 3
                if t == 0:
                    for bname, psz, _ in BLOCKS:
                        b3 = pbuf[bname][0].rearrange("p (r w) -> p r w", w=PW)
                        nc.vector.memset(b3[:, 0:1, 1:1 + W], 0.0)
                for bname, psz, ms in BLOCKS:
                    b3 = pbuf[bname][slot].rearrange("p (r w) -> p r w", w=PW)
                    for ci in range(2):
                        ps = mmpsum.tile([psz, 512], F32, tag="mmps", name="mmps")
                        nc.tensor.matmul(ps[:], wq_sb[0][:, ms:ms + psz],
                                         xt[0][:, ci * 512:(ci + 1) * 512],
                                         start=True, stop=False)
                        nc.tensor.matmul(ps[:], wq_sb[1][:, ms:ms + psz],
                                         xt[1][:, ci * 512:(ci + 1) * 512],
                                         start=False, stop=True)
                        # rows 1+4ci .. 5+4ci of padded buffer
                        nc.scalar.activation(
                            b3[:, 1 + 4 * ci:5 + 4 * ci, 1:1 + W],
                            ps[:].rearrange("p (r w) -> p r w", w=W), AF.Copy)
                        if ci == 0 and t > 0:
                            # halo: prev buffer row 9 <- this tile image row 0
                            pb_prev = pbuf[bname][(t - 1) % 3].rearrange(
                                "p (r w) -> p r w", w=PW)
                            nc.gpsimd.tensor_copy(
                                pb_prev[:, PR - 1:PR, 1:1 + W],
                                b3[:, 1:2, 1:1 + W])
                        if ci == 1 and t < NT - 1:
                            # halo: next buffer row 0 <- this tile image row 7
                            pb_next = pbuf[bname][(t + 1) % 3].rearrange(
                                "p (r w) -> p r w", w=PW)
                            nc.gpsimd.tensor_copy(
                                pb_next[:, 0:1, 1:1 + W],
                                b3[:, PR - 2:PR - 1, 1:1 + W])
                if t == NT - 1:
                    for bname, psz, _ in BLOCKS:
                        b3 = pbuf[bname][slot].rearrange("p (r w) -> p r w", w=PW)
                        nc.vector.memset(b3[:, PR - 1:PR, 1:1 + W], 0.0)

            OFFS = [(dy, dx) for dy in (-1, 0, 1) for dx in (-1, 0, 1)]

            def emit_stencil(t, ft):
                """stencil + f-mult + squares + transposes + gram for tile t."""
                slot = t % 3
                f0, f1 = ft
                qf = {}
                # --- per block stencil ---
                for bi, (bname, psz, _) in enumerate(BLOCKS):
                    b3 = pbuf[bname][slot].rearrange("p (r w) -> p r w", w=PW)
                    eng = stencil_engine(bname, t)
                    is_v = bname in ("v0", "v1")
                    fsrc = f1 if bname == "qk1" else f0
                    if not is_v:
                        q = qfpool.tile([psz, S], BF16, tag=f"qf_{bname}",
                                        name=f"qf_{bname}")
                        qf[bname] = q

                    def pe_chunk(ci):
                        sp = stpsum.tile([psz, 512], F32, tag="stps", name="stps")
                        for ti, (dy, dx) in enumerate(OFFS):
                            win = b3[:, 1 + 4 * ci + dy:5 + 4 * ci + dy,
                                     1 + dx:1 + dx + W]
                            if bname == "v1":
                                dg = diag64_sb[:, ti * 64:(ti + 1) * 64]
                            else:
                                dg = diag_sb[:, (bi * 9 + ti) * 128:
                                             (bi * 9 + ti) * 128 + psz]
                            nc.tensor.matmul(sp[:], dg, win,
                                             start=(ti == 0), stop=(ti == 8))
                        if is_v:
                            vi = 0 if bname == "v0" else 1
                            nc.scalar.activation(
                                v_sb[vi][:, t * S + ci * 512:
                                         t * S + (ci + 1) * 512],
                                sp[:], AF.Copy)
                        else:
                            nc.vector.tensor_mul(
                                qf[bname][:, ci * 512:(ci + 1) * 512], sp[:],
                                fsrc[:, ci * 512:(ci + 1) * 512])

                    def dve_chunks(cis):
                        nrow = 4 * len(cis)
                        r0_ = 4 * cis[0]
                        if is_v:
                            vi = 0 if bname == "v0" else 1
                            acc = v_sb[vi][:, t * S + r0_ * W:
                                           t * S + (r0_ + nrow) * W]
                        else:
                            acc = apool.tile([psz, 512 * len(cis)], BF16,
                                             tag=f"acc_{bname}{cis[0]}",
                                             name=f"acc_{bname}")
                        a3 = acc.rearrange("p (r w) -> p r w", w=W)
                        # 4x-mode tap products + 2x-mode add tree
                        prods = []
                        for ti, (dy, dx) in enumerate(OFFS):
                            win = b3[:, 1 + r0_ + dy:1 + r0_ + nrow + dy,
                                     1 + dx:1 + dx + W]
                            tap = taps_sb[0:psz, bi * 9 + ti:bi * 9 + ti + 1]
                            if ti == 0:
                                nc.vector.tensor_scalar_mul(a3[:], win, tap)
                            elif ti % 2 == 1:
                                pr = apool.tile([psz, 512 * len(cis)], BF16,
                                                tag=f"pr_{bname}{cis[0]}",
                                                name=f"pr_{bname}")
                                p3 = pr.rearrange("p (r w) -> p r w", w=W)
                                nc.vector.tensor_scalar_mul(p3[:], win, tap)
                                prods.append(p3)
                            else:
                                nc.vector.scalar_tensor_tensor(
                                    p3[:], win, tap, p3[:],
                                    op0=AL.mult, op1=AL.add)
                                nc.vector.tensor_add(a3[:], a3[:], p3[:])
                        # ti=7 leaves an un-accumulated product pair? no:
                        # pattern: ti 1(mul into p3) 2(stt+add) 3(mul) 4(stt+add)
                        # 5(mul) 6(stt+add) 7(mul) 8(stt+add) -> all folded
                        if not is_v:
                            nc.vector.tensor_mul(
                                qf[bname][:, r0_ * W:(r0_ + nrow) * W],
                                acc[:], fsrc[:, r0_ * W:(r0_ + nrow) * W])

                    if eng == "PE":
                        pe_chunk(0)
                        pe_chunk(1)
                    elif eng == "DVE":
                        dve_chunks((0, 1))
                    else:  # SPLIT
                        pe_chunk(0)
                        dve_chunks((1,))

                # --- squares (ACT) ---
                for nm in ("q0", "k0", "qk1"):
                    scr = scrpool.tile([128, S], BF16, tag=f"scr_{nm}",
                                       name=f"scr_{nm}")
                    nc.scalar.activation(scr[:], qf[nm][:], AF.Square,
                                         accum_out=sq_sb[nm][:, t:t + 1])

                # --- transposes (batched xbar DMA) ---
                qT = {}
                for nm in ("q0", "k0", "qk1"):
                    tt = trpool.tile([128, TR * 128], BF16, tag=f"T_{nm}",
                                     name=f"T_{nm}")
                    t3 = tt.rearrange("p (j c) -> p j c", c=128)
                    nc.sync.dma_start_transpose(t3[:], qf[nm][:])
                    qT[nm] = t3

                # --- gram accumulation ---
                for j in range(TR):
                    st = (t == 0 and j == 0)
                    sp = (t == NT - 1 and j == TR - 1)
                    q0c = qT["q0"][:, j, :]
                    q1c = qT["qk1"][:, j, 0:64]
                    k0c = qT["k0"][:, j, :]
                    k1c = qT["qk1"][:, j, 64:128]
                    nc.tensor.matmul(g_ps[0][:, 0:128], q0c, k0c,
                                     start=st, stop=sp)
                    nc.tensor.matmul(g_ps[0][:, 128:192], q0c, k1c,
                                     start=st, stop=sp)
                    nc.tensor.matmul(g_ps[1][:, 0:128], q1c, k0c,
                                     start=st, stop=sp)
                    nc.tensor.matmul(g_ps[1][:, 128:192], q1c, k1c,
                                     start=st, stop=sp)

            # ================= main pipelined loop =================
            prev = None
            for t in range(NT):
                xt, ft = emit_loads(t)
                emit_qkv(t, xt)
                if prev is not None:
                    emit_stencil(prev[0], prev[1])
                prev = (t, ft)
            emit_stencil(prev[0], prev[1])

            # ================= small-tensor phase =================
            r_ = {}
            for nm in ("q0", "k0", "qk1"):
                s1 = finpool.tile([128, 1], F32, tag=f"s1_{nm}", name=f"s1_{nm}")
                nc.vector.tensor_reduce(s1[:], sq_sb[nm][:],
                                        axis=mybir.AxisListType.X, op=AL.add)
                nc.vector.tensor_scalar_max(s1[:], s1[:], 1e-24)
                sq1 = finpool.tile([128, 1], F32, tag=f"sr_{nm}", name=f"sr_{nm}")
                nc.scalar.activation(sq1[:], s1[:], AF.Sqrt)
                rr = finpool.tile([128, 1], F32, tag=f"r_{nm}", name=f"r_{nm}")
                nc.vector.reciprocal(rr[:], sq1[:])
                r_[nm] = rr

            G_sb = [finpool.tile([128, C], F32, tag="G0", name="G0"),
                    finpool.tile([64, C], F32, tag="G1", name="G1")]
            nc.vector.tensor_scalar_mul(G_sb[0][:], g_ps[0][:], r_["q0"][:])
            nc.vector.tensor_scalar_mul(G_sb[1][:], g_ps[1][:], r_["qk1"][0:64, :])

            rkf = finpool.tile([1, C], F32, tag="rkf", name="rkf")
            nc.gpsimd.dma_start(out=rkf[0:1, 0:128], in_=r_["k0"][:, 0:1])
            nc.gpsimd.dma_start(out=rkf[0:1, 128:192], in_=r_["qk1"][64:128, 0:1])
            nc.vector.tensor_mul(rkf[:], rkf[:], tmpf_sb[:])
            rkb = finpool.tile([128, C], F32, tag="rkb", name="rkb")
            nc.gpsimd.partition_broadcast(rkb[:], rkf[:])
            nc.vector.tensor_mul(G_sb[0][:], G_sb[0][:], rkb[0:128, :])
            nc.vector.tensor_mul(G_sb[1][:], G_sb[1][:], rkb[0:64, :])

            # extract per-head 24x24 blocks: at[c, h*24+d] = Gs[24h+c, 24h+d]
            at = finpool.tile([CH, HEADS * CH], F32, tag="at", name="at")
            for h in range(HEADS):
                a0 = h * CH
                col = slice(a0, a0 + CH)
                dst = at[:, a0:a0 + CH]
                if a0 + CH <= 128:
                    nc.sync.dma_start(out=dst, in_=G_sb[0][a0:a0 + CH, col])
                elif a0 >= 128:
                    nc.sync.dma_start(out=dst,
                                      in_=G_sb[1][a0 - 128:a0 - 128 + CH, col])
                else:
                    m = 128 - a0
                    nc.sync.dma_start(out=at[0:m, a0:a0 + CH],
                                      in_=G_sb[0][a0:128, col])
                    nc.sync.dma_start(out=at[m:CH, a0:a0 + CH],
                                      in_=G_sb[1][0:a0 + CH - 128, col])

            e_sb = finpool.tile([CH, HEADS * CH], F32, tag="e", name="e")
            nc.scalar.activation(e_sb[:], at[:], AF.Exp)
            e3 = e_sb.rearrange("p (h d) -> p h d", d=CH)
            sums = finpool.tile([CH, HEADS], F32, tag="sums", name="sums")
            nc.vector.tensor_reduce(sums[:], e3[:], axis=mybir.AxisListType.X,
                                    op=AL.add)
            rs = finpool.tile([CH, HEADS], F32, tag="rs", name="rs")
            nc.vector.reciprocal(rs[:], sums[:])
            attn = finpool.tile([CH, HEADS * CH], BF16, tag="attn", name="attn")
            for h in range(HEADS):
                nc.vector.tensor_scalar_mul(
                    attn[:, h * CH:(h + 1) * CH],
                    e_sb[:, h * CH:(h + 1) * CH], rs[:, h:h + 1])

            # blockdiag(attn) as two 96-row contraction blocks
            bd = [finpool.tile([96, C], BF16, tag="bd0", name="bd0"),
                  finpool.tile([96, C], BF16, tag="bd1", name="bd1")]
            nc.vector.memset(bd[0][:], 0.0)
            nc.vector.memset(bd[1][:], 0.0)
            for h in range(HEADS):
                nc.sync.dma_start(
                    out=bd[h // 4][(h % 4) * CH:(h % 4) * CH + CH,
                                   h * CH:(h + 1) * CH],
                    in_=attn[:, h * CH:(h + 1) * CH])
            mtps_all = fpsum.tile([128, 512], F32, tag="mtps", name="mtps")
            mt_ps = [mtps_all[:, 0:C], mtps_all[0:64, C:2 * C]]
            for mi, msl in enumerate((slice(0, 128), slice(128, 192))):
                for k in range(2):
                    nc.tensor.matmul(mt_ps[mi][:], bd[k][:, msl], wp_sb[k][:],
                                     start=(k == 0), stop=(k == 1))
            mt_sb = [finpool.tile([128, C], BF16, tag="mts0", name="mts0"),
                     finpool.tile([64, C], BF16, tag="mts1", name="mts1")]
            nc.vector.tensor_copy(mt_sb[0][:], mt_ps[0][:])
            nc.vector.tensor_copy(mt_sb[1][:], mt_ps[1][:])

            # ---- output: out = MT^T @ v ----
            for jj in range(N // 1024):
                osb = [outpool.tile([128, 1024], BF16, tag="osb0", name="osb0"),
                       outpool.tile([64, 1024], BF16, tag="osb1", name="osb1")]
                for half in range(2):
                    col = slice(jj * 1024 + half * 512, jj * 1024 + (half + 1) * 512)
                    ocol = slice(half * 512, (half + 1) * 512)
                    for mi, (msz, msl) in enumerate(((128, slice(0, 128)),
                                                     (64, slice(128, 192)))):
                        ps = mmpsum.tile([msz, 512], F32, tag="mmps", name="mmps")
                        nc.tensor.matmul(ps[:], mt_sb[0][:, msl],
                                         v_sb[0][:, col], start=True, stop=False)
                        nc.tensor.matmul(ps[:], mt_sb[1][:, msl],
                                         v_sb[1][:, col], start=False, stop=True)
                        if (jj + half + mi) % 2 == 0:
                            nc.scalar.activation(osb[mi][:, ocol], ps[:], AF.Copy)
                        else:
                            nc.vector.tensor_copy(osb[mi][:, ocol], ps[:])
                nc.gpsimd.dma_start(out=out2[0:128, jj * 1024:(jj + 1) * 1024],
                                    in_=osb[0][:])
                nc.gpsimd.dma_start(out=out2[128:192, jj * 1024:(jj + 1) * 1024],
                                    in_=osb[1][:])
    nc.finalize()
    return nc


_NC_CACHE = {}

# reordered qkv output channel index lists (in original 0..575 space)
_ORDER = (list(range(0, 128))          # q0
          + list(range(192, 320))      # k0
          + list(range(384, 512))      # v0
          + list(range(128, 192)) + list(range(320, 384))   # qk1 stacked
          + list(range(512, 576)))     # v1


def _prep_consts(W_qkv, W_dw, W_proj, temperature):
    import ml_dtypes
    order = np.array(_ORDER)
    wq = np.asarray(W_qkv, np.float32)[order, :]          # [576, 192] reordered
    wqT = np.ascontiguousarray(wq.T).astype(ml_dtypes.bfloat16)
    w9 = np.asarray(W_dw, np.float32).reshape(O, 9)[order, :]   # [576, 9]
    # taps param [128, 45]: col b*9+t = tap t of block b (on that block's partitions)
    taps = np.zeros((128, NB * 9), np.float32)
    starts = [0, 128, 256, 384, 512]
    for b in range(NB):
        psz = 64 if b == 4 else 128
        taps[0:psz, b * 9:(b + 1) * 9] = w9[starts[b]:starts[b] + psz, :]
    # diag matrices
    d128 = np.zeros((128, 4 * 9 * 128), np.float32)
    for b in range(4):
        for t in range(9):
            d128[:, (b * 9 + t) * 128:(b * 9 + t + 1) * 128] = np.diag(
                w9[starts[b]:starts[b] + 128, t])
    d64 = np.zeros((64, 9 * 64), np.float32)
    for t in range(9):
        d64[:, t * 64:(t + 1) * 64] = np.diag(w9[512:576, t])
    wp = np.ascontiguousarray(np.asarray(W_proj, np.float32).T).astype(
        ml_dtypes.bfloat16)
    tmpf = np.repeat(np.asarray(temperature, np.float32).reshape(HEADS), CH)
    return {
        "wq": wqT,
        "taps": np.ascontiguousarray(taps),
        "diag128": np.ascontiguousarray(d128.astype(ml_dtypes.bfloat16)),
        "diag64": np.ascontiguousarray(d64.astype(ml_dtypes.bfloat16)),
        "wp": wp,
        "tmpf": np.ascontiguousarray(tmpf.reshape(1, C)),
    }


def kernel(x, feature, W_qkv, W_dw, W_proj, temperature):
    b = x.shape[0]
    consts = _prep_consts(W_qkv, W_dw, W_proj, temperature)

    if "nc" not in _NC_CACHE:
        _NC_CACHE["nc"] = build_nc()
    nc = _NC_CACHE["nc"]

    in_maps = []
    for i in range(b):
        m = {"x": np.ascontiguousarray(np.asarray(x[i], np.float32)),
             "f": np.ascontiguousarray(np.asarray(feature[i], np.float32))}
        m.update(consts)
        in_maps.append(m)
    res = run_bass_kernel_spmd(nc, in_maps, list(range(b)))
    _NC_CACHE["last_result"] = res
    outs = [np.asarray(r["out"], np.float32).reshape(C, H, W)
            for r in res.results]
    return np.stack(outs, axis=0)
